# revision 1
# baseline (speedup 1.0000x reference)
"""AttnBlock (GroupNorm + 4096-token single-head attention + residual) on 8 trn2 cores.

Sharding: 2 cores per batch sample. Each core computes GroupNorm + K/V for the
full sample (duplicated within the pair) and attention for half the queries
(2048 of 4096). A single SPMD program serves both halves: the host rotates the
sample's spatial columns so each core's query half always sits at columns
0..2047 (attention is permutation-invariant over keys; GroupNorm stats are
permutation-invariant over spatial positions).

Host-side prep (data marshaling only): weights are pre-transposed to the
[c_in, c_out] stationary-operand layout and pre-cast to bf16; bias/affine
vectors are pre-arranged into [128, 4] per-partition column layout.

Compute layout (per core):
  h = GN(x) [C=512, N=4096] bf16   (stats: DVE sum-reduce + ACT Square+accum)
  k = wk @ h [C, N] bf16;  q = wq @ h[:, :2048] [C, 2048] bf16
  vT = h^T @ wv^T [N, C] bf16  (natural matmul output layout, no transposes)
  scores^T[nk, q] = k^T q  (softmax denominator and PV both want nk on
  partitions; softmax itself needs no max-subtraction: scores ~ N(0,1))
  e = exp(scores * C^-0.5) bf16     (1-deep software pipeline vs PV)
  out_T[c, q] += v^T[nk,c].T @ e ; denom[1, q] += ones[nk,1].T @ e
  proj = wo @ out_T ; out = proj * (1/denom bcast) + bo + residual
  (softmax division deferred past the projection so the PV accumulators
  release after a plain bf16 cast and the reciprocal chain overlaps proj)
"""

import sys

for _p in ("/opt/trn_rl_repo", "/root/.axon_site/_ro/trn_rl_repo"):
    if _p not in sys.path:
        sys.path.append(_p)

import ml_dtypes
import numpy as np

C = 512
N = 4096
NQ = 2048
P = 128
CT = C // P  # 4 c-tiles
NKB = N // P  # 32 nk blocks
QCH = NQ // 512  # 4 q chunks of 512
EPS = 1e-5
SCALE = float(C) ** -0.5

_cache = {}


def _build():
    import concourse.bacc as bacc
    import concourse.bass as bass
    import concourse.mybir as mybir
    import concourse.tile as tile
    from concourse.masks import make_identity

    f32 = mybir.dt.float32
    bf16 = mybir.dt.bfloat16
    AF = mybir.ActivationFunctionType
    ALU = mybir.AluOpType
    AX = mybir.AxisListType

    nc = bacc.Bacc("TRN2", target_bir_lowering=False, debug=False, num_devices=8)

    x_d = nc.dram_tensor("x", [C, N], f32, kind="ExternalInput")
    wT_d = {
        nm: nc.dram_tensor(nm, [C, C], bf16, kind="ExternalInput")
        for nm in ("wqT", "wkT", "wvT", "woT")
    }
    col_d = {
        nm: nc.dram_tensor(nm, [P, CT], f32, kind="ExternalInput")
        for nm in ("bqc", "bkc", "boc", "gnwc", "gnbc")
    }
    bvb_d = nc.dram_tensor("bvb", [P, C], f32, kind="ExternalInput")
    out_d = nc.dram_tensor("out", [C, NQ], f32, kind="ExternalOutput")

    x_t = x_d.ap().rearrange("(t p) n -> t p n", p=P)
    out_t = out_d.ap().rearrange("(t p) n -> t p n", p=P)

    with tile.TileContext(nc) as tc:
        with (
            tc.tile_pool(name="const", bufs=1) as const,
            tc.tile_pool(name="work", bufs=3) as work,
            tc.tile_pool(name="wtp", bufs=1) as wtp,
            tc.tile_pool(name="hp", bufs=1) as hp,
            tc.tile_pool(name="ps_s", bufs=2, space="PSUM") as ps_s,
            tc.tile_pool(name="ps_o", bufs=4, space="PSUM") as ps_o,
            tc.tile_pool(name="ps_d", bufs=2, space="PSUM") as ps_d,
        ):
            # ---- constants ----
            ident = const.tile([P, P], f32)
            make_identity(nc, ident)
            ones_mat = const.tile([P, P], bf16)
            nc.vector.memset(ones_mat, 1.0)
            ones_row = const.tile([1, P], f32)
            nc.vector.memset(ones_row, 1.0)
            eps_t = const.tile([P, 1], f32)
            nc.vector.memset(eps_t, EPS)

            cols = {}
            for nm in ("bqc", "bkc", "boc", "gnwc", "gnbc"):
                cols[nm] = const.tile([P, CT], f32, tag=f"c_{nm}", name=f"c_{nm}")
                nc.scalar.dma_start(cols[nm], col_d[nm].ap())
            bv_bcast = const.tile([P, C], f32)
            nc.scalar.dma_start(bv_bcast, bvb_d.ap())

            # x on the sync queue; pre-transposed bf16 weights on the scalar
            # queue (both stream concurrently)
            wT = {}
            for nm in ("wkT", "wqT", "wvT", "woT"):
                wT[nm] = []
                w_tiled = wT_d[nm].ap().rearrange("(t p) c -> t p c", p=P)
                for ci in range(CT):
                    wt = wtp.tile([P, C], bf16, tag=f"{nm}{ci}", name=f"{nm}{ci}")
                    nc.scalar.dma_start(wt, w_tiled[ci])
                    wT[nm].append(wt)

            ht = [hp.tile([P, N], bf16, tag=f"h{t}", name=f"h{t}") for t in range(CT)]

            with tc.tile_pool(name="xp", bufs=1) as xp:
                xt = []
                for t in range(CT):
                    xtile = xp.tile([P, N], f32, tag=f"x{t}", name=f"x{t}")
                    nc.sync.dma_start(xtile, x_t[t])
                    xt.append(xtile)

                # ---- GroupNorm stats ----
                # per-channel SUM (DVE reduce) and SUM of squares (ACT Square
                # with accumulate) -> PE transpose -> group-sum (16 consecutive
                # channels per group land in one row after transpose) ->
                # broadcast back -> per-channel scale/bias columns.
                mvpack = const.tile([P, 2 * CT], f32)
                for t in range(CT):
                    nc.vector.tensor_reduce(
                        out=mvpack[:, t : t + 1], in_=xt[t], axis=AX.X, op=ALU.add
                    )
                    junk = xp.tile([P, N], bf16, tag="junk", bufs=2)
                    nc.scalar.activation(
                        out=junk,
                        in_=xt[t],
                        func=AF.Square,
                        accum_out=mvpack[:, CT + t : CT + t + 1],
                    )

                pst1 = ps_o.tile([8, P], f32, tag="o", name="pst1")
                nc.tensor.transpose(pst1, mvpack, ident)
                statsT = const.tile([8, P], f32)
                nc.vector.tensor_copy(statsT, pst1)
                gsum = const.tile([8, 8], f32)
                nc.vector.tensor_reduce(
                    out=gsum,
                    in_=statsT.rearrange("p (g s) -> p g s", s=16),
                    axis=AX.X,
                    op=ALU.add,
                )
                nc.vector.tensor_scalar_mul(gsum, gsum, 1.0 / (16.0 * 4096.0))
                bcast16 = const.tile([8, P], f32)
                gsum_rep = bass.AP(
                    tensor=gsum.tensor,
                    offset=gsum.offset,
                    ap=list(gsum.ap) + [[0, 16]],
                )
                nc.vector.tensor_copy(
                    out=bcast16.rearrange("p (g s) -> p g s", s=16), in_=gsum_rep
                )
                pst2 = ps_o.tile([P, 8], f32, tag="o", name="pst2")
                nc.tensor.transpose(pst2, bcast16, ident[:8, :8])
                gcols = const.tile([P, 2 * CT], f32)
                nc.vector.tensor_copy(gcols, pst2)

                var_c = const.tile([P, CT], f32)
                nc.vector.tensor_mul(var_c, gcols[:, 0:CT], gcols[:, 0:CT])
                nc.vector.tensor_sub(var_c, gcols[:, CT : 2 * CT], var_c)
                rstd_c = const.tile([P, CT], f32)
                nc.scalar.activation(out=rstd_c, in_=var_c, func=AF.Sqrt, bias=eps_t)
                nc.vector.reciprocal(rstd_c, rstd_c)
                scale_c = const.tile([P, CT], f32)
                nc.vector.tensor_mul(scale_c, rstd_c, cols["gnwc"])
                bias_c = const.tile([P, CT], f32)
                nc.vector.tensor_mul(bias_c, gcols[:, 0:CT], scale_c)
                nc.vector.tensor_sub(bias_c, cols["gnbc"], bias_c)

                # ---- h = GN(x) bf16, split across DVE and ACT ----
                for t in range(CT):
                    if t < 2:
                        nc.vector.tensor_scalar(
                            out=ht[t],
                            in0=xt[t],
                            scalar1=scale_c[:, t : t + 1],
                            scalar2=bias_c[:, t : t + 1],
                            op0=ALU.mult,
                            op1=ALU.add,
                        )
                    else:
                        nc.scalar.activation(
                            out=ht[t],
                            in_=xt[t],
                            func=AF.Identity,
                            bias=bias_c[:, t : t + 1],
                            scale=scale_c[:, t : t + 1],
                        )
            # xp closed: x space freed

            with (
                tc.tile_pool(name="kqv", bufs=1) as kqv,
                tc.tile_pool(name="etp", bufs=1) as etp,
            ):
                kt = [
                    kqv.tile([P, N], bf16, tag=f"k{t}", name=f"k{t}")
                    for t in range(CT)
                ]
                qt = [
                    kqv.tile([P, NQ], bf16, tag=f"q{t}", name=f"q{t}")
                    for t in range(CT)
                ]
                # ---- k [C, N], q [C, NQ] ----
                for t in range(CT):
                    for nb in range(N // 512):
                        ps = ps_s.tile([P, 512], f32, tag="s")
                        for ci in range(CT):
                            nc.tensor.matmul(
                                ps,
                                lhsT=wT["wkT"][ci][:, t * P : (t + 1) * P],
                                rhs=ht[ci][:, nb * 512 : (nb + 1) * 512],
                                start=(ci == 0),
                                stop=(ci == CT - 1),
                            )
                        nc.scalar.activation(
                            out=kt[t][:, nb * 512 : (nb + 1) * 512],
                            in_=ps,
                            func=AF.Identity,
                            bias=cols["bkc"][:, t : t + 1],
                        )
                for t in range(CT):
                    for nb in range(NQ // 512):
                        ps = ps_s.tile([P, 512], f32, tag="s")
                        for ci in range(CT):
                            nc.tensor.matmul(
                                ps,
                                lhsT=wT["wqT"][ci][:, t * P : (t + 1) * P],
                                rhs=ht[ci][:, nb * 512 : (nb + 1) * 512],
                                start=(ci == 0),
                                stop=(ci == CT - 1),
                            )
                        nc.scalar.activation(
                            out=qt[t][:, nb * 512 : (nb + 1) * 512],
                            in_=ps,
                            func=AF.Identity,
                            bias=cols["bqc"][:, t : t + 1],
                        )

                # ---- vT [N, C] ----
                vt = []
                for nb in range(NKB):
                    ps = ps_o.tile([P, 512], f32, tag="o")
                    for ci in range(CT):
                        nc.tensor.matmul(
                            ps,
                            lhsT=ht[ci][:, nb * P : (nb + 1) * P],
                            rhs=wT["wvT"][ci],
                            start=(ci == 0),
                            stop=(ci == CT - 1),
                        )
                    v = kqv.tile([P, C], bf16, tag=f"vt{nb}", name=f"vt{nb}")
                    nc.vector.tensor_add(out=v, in0=ps, in1=bv_bcast)
                    vt.append(v)

                # ---- attention ----
                # Two PE-dense phases per q-chunk with chunk-resident exp
                # tiles: (1) all scores + Exp evictions, (2) all PV + denom
                # matmuls with zero ACT dependencies. Chunks are software-
                # pipelined: the next chunk's scores phase is emitted before
                # this chunk's epilogue (casts/reciprocal/proj), so the PE
                # crosses every chunk boundary with independent matmuls.
                def scores_phase(qc):
                    qs = qc * 512
                    ets = []
                    for j in range(NKB):
                        pss = ps_s.tile([P, 512], f32, tag="s", name="pss")
                        for ci in range(CT):
                            nc.tensor.matmul(
                                pss,
                                lhsT=kt[ci][:, j * P : (j + 1) * P],
                                rhs=qt[ci][:, qs : qs + 512],
                                start=(ci == 0),
                                stop=(ci == CT - 1),
                            )
                        et = etp.tile([P, 512], bf16, tag=f"et{j}", name=f"et{j}")
                        nc.scalar.activation(out=et, in_=pss, func=AF.Exp, scale=SCALE)
                        ets.append(et)
                    return ets

                def pv_phase(ets):
                    pso = [
                        ps_o.tile([P, 512], f32, tag="o", name="pso")
                        for _ in range(CT)
                    ]
                    psd = ps_d.tile([P, 512], f32, tag="d")
                    for j in range(NKB):
                        for co in range(CT):
                            nc.tensor.matmul(
                                pso[co],
                                lhsT=vt[j][:, co * P : (co + 1) * P],
                                rhs=ets[j],
                                start=(j == 0),
                                stop=(j == NKB - 1),
                            )
                        nc.tensor.matmul(
                            psd,
                            lhsT=ones_mat,
                            rhs=ets[j],
                            start=(j == 0),
                            stop=(j == NKB - 1),
                        )
                    return pso, psd

                def epilogue(qc, pso, psd):
                    qs = qc * 512
                    aoT = []
                    for co in range(CT):
                        a = work.tile([P, 512], bf16, tag=f"ao{co}", bufs=1, name="ao")
                        nc.vector.tensor_copy(a, pso[co])
                        aoT.append(a)
                    dsb = work.tile([1, 512], f32, tag="dsb", bufs=2)
                    nc.vector.tensor_copy(dsb, psd[0:1, :])
                    psb = ps_d.tile([P, 512], f32, tag="d", name="psb")
                    nc.tensor.matmul(
                        psb, lhsT=ones_row, rhs=dsb, start=True, stop=True
                    )
                    rdb = work.tile([P, 512], f32, tag="rdb", bufs=2)
                    nc.vector.reciprocal(rdb, psb)
                    for co in range(CT):
                        xres = work.tile([P, 512], f32, tag="xres", bufs=4)
                        nc.sync.dma_start(xres, x_t[co][:, qs : qs + 512])
                        nc.vector.tensor_scalar_add(
                            out=xres, in0=xres, scalar1=cols["boc"][:, co : co + 1]
                        )
                        psp = ps_d.tile([P, 512], f32, tag="d", name="psp")
                        for ci in range(CT):
                            nc.tensor.matmul(
                                psp,
                                lhsT=wT["woT"][ci][:, co * P : (co + 1) * P],
                                rhs=aoT[ci],
                                start=(ci == 0),
                                stop=(ci == CT - 1),
                            )
                        osb = work.tile([P, 512], f32, tag="osb", bufs=3)
                        nc.vector.tensor_mul(osb, psp, rdb)
                        nc.vector.tensor_add(out=osb, in0=osb, in1=xres)
                        nc.sync.dma_start(out_t[co][:, qs : qs + 512], osb)

                ets = scores_phase(0)
                for qc in range(QCH):
                    pso, psd = pv_phase(ets)
                    ets = scores_phase(qc + 1) if qc + 1 < QCH else None
                    epilogue(qc, pso, psd)

    nc.compile()
    return nc


def _get_nc():
    if "nc" not in _cache:
        _cache["nc"] = _build()
    return _cache["nc"]


def _prep_common(inputs):
    bf16 = ml_dtypes.bfloat16

    def colize(v):
        v = np.asarray(v, np.float32).reshape(CT, P)
        return np.ascontiguousarray(v.T)

    return {
        "wqT": np.ascontiguousarray(np.asarray(inputs["wq"], np.float32).T.astype(bf16)),
        "wkT": np.ascontiguousarray(np.asarray(inputs["wk"], np.float32).T.astype(bf16)),
        "wvT": np.ascontiguousarray(np.asarray(inputs["wv"], np.float32).T.astype(bf16)),
        "woT": np.ascontiguousarray(np.asarray(inputs["wo"], np.float32).T.astype(bf16)),
        "bqc": colize(inputs["bq"]),
        "bkc": colize(inputs["bk"]),
        "boc": colize(inputs["bo"]),
        "gnwc": colize(inputs["gn_w"]),
        "gnbc": colize(inputs["gn_b"]),
        "bvb": np.ascontiguousarray(
            np.tile(np.asarray(inputs["bv"], np.float32)[None, :], (P, 1))
        ),
    }


def make_in_maps(inputs):
    x = np.ascontiguousarray(np.asarray(inputs["hidden_states"], dtype=np.float32))
    B = x.shape[0]
    xs = x.reshape(B, C, N)
    common = _prep_common(inputs)
    in_maps = []
    for core in range(8):
        s, half = core // 2, core % 2
        xc = xs[s] if half == 0 else np.ascontiguousarray(np.roll(xs[s], -NQ, axis=1))
        in_maps.append({"x": xc, **common})
    return in_maps


def kernel(**inputs):
    from concourse.bass_utils import run_bass_kernel_spmd

    nc = _get_nc()
    in_maps = make_in_maps(inputs)
    res = run_bass_kernel_spmd(nc, in_maps, list(range(8)))

    B = np.asarray(inputs["hidden_states"]).shape[0]
    out = np.empty((B, C, N), np.float32)
    for core in range(8):
        s, half = core // 2, core % 2
        out[s][:, half * NQ : (half + 1) * NQ] = res.results[core]["out"]
    return out.reshape(B, C, 64, 64)



# revision 6
# speedup vs baseline: 1.7916x; 1.7916x over previous
"""AttnBlock (GroupNorm + 4096-token single-head attention + residual) on 8 trn2 cores.

Sharding: 2 cores per batch sample. Each core computes GroupNorm + K/V for the
full sample (duplicated within the pair) and attention for half the queries
(2048 of 4096). A single SPMD program serves both halves: the host rotates the
sample's spatial columns so each core's query half always sits at columns
0..2047 (attention is permutation-invariant over keys; GroupNorm stats are
permutation-invariant over spatial positions).

All heavy matmuls run in fp8(e4m3) with perf_mode=DoubleRow (K=256 per pass,
2x PE throughput vs bf16). The softmax exp is computed as exp(s*scale - 2.5)
so the fp8 range (TRN e4m3 max = 240) is never exceeded; the constant factor
cancels between the PV numerator and the ones-matmul denominator at the final
division. The residual path stays exact fp32 (separate DMA of the fp32 input
columns); GroupNorm stats come from a bf16 copy of x (host-cast) and are
computed per 128-channel tile (each 16-channel group lives inside one tile),
pipelined with the x DMA so the PE starts ~17us into the kernel.

Host-side prep (data marshaling only): weights pre-arranged to the
[128, cin_tile, cout] fp8 DoubleRow layout; bias/affine vectors to [128, 4]
per-partition columns; x pre-cast to bf16 (stats/value path) alongside the
fp32 query-half columns (residual path).
"""

import sys

for _p in ("/opt/trn_rl_repo", "/root/.axon_site/_ro/trn_rl_repo"):
    if _p not in sys.path:
        sys.path.append(_p)

import ml_dtypes
import numpy as np

C = 512
N = 4096
NQ = 2048
P = 128
CT = C // P  # 4 c-tiles
NKB = N // P  # 32 nk blocks
QCH = NQ // 512  # 4 q chunks of 512
EPS = 1e-5
SCALE = float(C) ** -0.5
EXP_BIAS = -2.5  # exp(s*SCALE - 2.5): keeps fp8 et <= ~25; cancels in division
AO_SCALE = 1.0 / 16.0  # unnormalized PV sums reach ~550; scale into fp8 range
GN_INV = 1.0 / (16.0 * 4096.0)

_cache = {}


def _build():
    import concourse.bacc as bacc
    import concourse.bass as bass
    import concourse.mybir as mybir
    import concourse.tile as tile

    f32 = mybir.dt.float32
    bf16 = mybir.dt.bfloat16
    fp8 = mybir.dt.float8e4
    AF = mybir.ActivationFunctionType
    ALU = mybir.AluOpType
    AX = mybir.AxisListType
    DR = mybir.MatmulPerfMode.DoubleRow

    nc = bacc.Bacc("TRN2", target_bir_lowering=False, debug=False, num_devices=8)

    xb_d = nc.dram_tensor("xb", [C, N], bf16, kind="ExternalInput")
    xr_d = nc.dram_tensor("xr", [C, NQ], f32, kind="ExternalInput")
    w3_d = {
        nm: nc.dram_tensor(nm, [P, CT, C], fp8, kind="ExternalInput")
        for nm in ("wq3", "wk3", "wv3", "wo3")
    }
    col_d = {
        nm: nc.dram_tensor(nm, [P, CT], f32, kind="ExternalInput")
        for nm in ("bqc", "bkc", "boc", "gnwc", "gnbc")
    }
    bvb_d = nc.dram_tensor("bvb", [P, C], f32, kind="ExternalInput")
    g_d = nc.dram_tensor("gmat", [P, 8], bf16, kind="ExternalInput")
    gt_d = nc.dram_tensor("gtmat", [8, P], bf16, kind="ExternalInput")
    out_d = nc.dram_tensor("out", [C, NQ], f32, kind="ExternalOutput")

    xb_t = xb_d.ap().rearrange("(t p) n -> t p n", p=P)
    xr_t = xr_d.ap().rearrange("(t p) n -> t p n", p=P)
    out_t = out_d.ap().rearrange("(t p) n -> t p n", p=P)

    with tile.TileContext(nc) as tc:
        with (
            tc.tile_pool(name="const", bufs=1) as const,
            tc.tile_pool(name="work", bufs=3) as work,
            tc.tile_pool(name="wtp", bufs=1) as wtp,
            tc.tile_pool(name="hp", bufs=1) as hp,
            tc.tile_pool(name="ps_s", bufs=2, space="PSUM") as ps_s,
            tc.tile_pool(name="ps_o", bufs=4, space="PSUM") as ps_o,
            tc.tile_pool(name="ps_d", bufs=2, space="PSUM") as ps_d,
        ):
            # ---- constants ----
            ones3 = const.tile([P, 2, P], fp8)
            nc.vector.memset(ones3, 1.0)
            eps8 = const.tile([8, 1], f32)
            nc.vector.memset(eps8, EPS)
            ebias = const.tile([P, 1], f32)
            nc.vector.memset(ebias, EXP_BIAS)
            gmat = const.tile([P, 8], bf16)
            nc.scalar.dma_start(gmat, g_d.ap())
            gtmat = const.tile([8, P], bf16)
            nc.scalar.dma_start(gtmat, gt_d.ap())

            cols = {}
            for nm in ("bqc", "bkc", "boc", "gnwc", "gnbc"):
                cols[nm] = const.tile([P, CT], f32, tag=f"c_{nm}", name=f"c_{nm}")
                nc.scalar.dma_start(cols[nm], col_d[nm].ap())
            bv_bcast = const.tile([P, C], f32)
            nc.scalar.dma_start(bv_bcast, bvb_d.ap())

            # x (bf16) on the sync queue; fp8 weights on the scalar queue
            w3 = {}
            for nm in ("wk3", "wq3", "wv3", "wo3"):
                w3[nm] = wtp.tile([P, CT, C], fp8, tag=nm, name=nm)
                nc.scalar.dma_start(w3[nm], w3_d[nm].ap())

            h_all = hp.tile([P, CT, N], fp8)

            with tc.tile_pool(name="xp", bufs=1) as xp:
                for t in range(CT):
                    xt = xp.tile([P, N], bf16, tag=f"x{t}", name=f"x{t}")
                    nc.sync.dma_start(xt, xb_t[t])

                    # ---- per-tile GroupNorm stats ----
                    # (8 groups of 16 channels live entirely in this tile)
                    st2 = work.tile([P, 2], f32, tag="st2", bufs=2)
                    nc.vector.tensor_reduce(
                        out=st2[:, 0:1], in_=xt, axis=AX.X, op=ALU.add
                    )
                    junk = xp.tile([P, N], bf16, tag="junk", bufs=2)
                    nc.scalar.activation(
                        out=junk, in_=xt, func=AF.Square, accum_out=st2[:, 1:2]
                    )
                    st2b = work.tile([P, 2], bf16, tag="st2b", bufs=2)
                    nc.vector.tensor_copy(st2b, st2)
                    psg = ps_d.tile([8, 2], f32, tag="d", name="psg")
                    nc.tensor.matmul(psg, lhsT=gmat, rhs=st2b, start=True, stop=True)
                    # [8,2] per-group (sum, sumsq) -> (mean, rstd)
                    sb8 = work.tile([8, 2], f32, tag="sb8", bufs=2)
                    nc.scalar.mul(sb8, psg, GN_INV)
                    var8 = work.tile([8, 1], f32, tag="var8", bufs=2)
                    nc.vector.tensor_mul(var8, sb8[:, 0:1], sb8[:, 0:1])
                    nc.vector.tensor_sub(var8, sb8[:, 1:2], var8)
                    nc.scalar.activation(
                        out=var8, in_=var8, func=AF.Sqrt, bias=eps8
                    )
                    mr8 = work.tile([8, 2], bf16, tag="mr8", bufs=2)
                    nc.vector.tensor_copy(mr8[:, 0:1], sb8[:, 0:1])
                    nc.vector.reciprocal(var8, var8)
                    nc.vector.tensor_copy(mr8[:, 1:2], var8)
                    psc = ps_d.tile([P, 2], f32, tag="d", name="psc")
                    nc.tensor.matmul(psc, lhsT=gtmat, rhs=mr8, start=True, stop=True)
                    scale_c = work.tile([P, 1], f32, tag="scale_c", bufs=2)
                    nc.vector.tensor_mul(scale_c, psc[:, 1:2], cols["gnwc"][:, t : t + 1])
                    bias_c = work.tile([P, 1], f32, tag="bias_c", bufs=2)
                    nc.vector.tensor_mul(bias_c, psc[:, 0:1], scale_c)
                    nc.vector.tensor_sub(bias_c, cols["gnbc"][:, t : t + 1], bias_c)

                    # ---- h tile = GN(x) fp8, split across DVE and ACT ----
                    nc.vector.tensor_scalar(
                        out=h_all[:, t, : N // 2],
                        in0=xt[:, : N // 2],
                        scalar1=scale_c,
                        scalar2=bias_c,
                        op0=ALU.mult,
                        op1=ALU.add,
                    )
                    nc.scalar.activation(
                        out=h_all[:, t, N // 2 :],
                        in_=xt[:, N // 2 :],
                        func=AF.Identity,
                        bias=bias_c,
                        scale=scale_c,
                    )
            # xp closed: x space freed

            with (
                tc.tile_pool(name="kqv", bufs=1) as kqv,
                tc.tile_pool(name="etp", bufs=1) as etp,
            ):
                k_all = kqv.tile([P, CT, N], fp8)
                q_all = kqv.tile([P, CT, NQ], fp8)
                v_all = kqv.tile([P, NKB, C], fp8)
                ao_all = kqv.tile([P, CT, 512], fp8)
                et2 = [
                    etp.tile([P, 2, 512], fp8, tag=f"et{jj}", name=f"et{jj}")
                    for jj in range(NKB // 2)
                ]

                # ---- k [C, N], q [C, NQ] (DoubleRow over c-pairs) ----
                for t in range(CT):
                    for nb in range(N // 512):
                        ps = ps_s.tile([P, 512], f32, tag="s")
                        for i2 in range(2):
                            nc.tensor.matmul(
                                ps,
                                lhsT=w3["wk3"][:, 2 * i2 : 2 * i2 + 2, t * P : (t + 1) * P],
                                rhs=h_all[:, 2 * i2 : 2 * i2 + 2, nb * 512 : (nb + 1) * 512],
                                start=(i2 == 0),
                                stop=(i2 == 1),
                                perf_mode=DR,
                            )
                        nc.scalar.activation(
                            out=k_all[:, t, nb * 512 : (nb + 1) * 512],
                            in_=ps,
                            func=AF.Identity,
                            bias=cols["bkc"][:, t : t + 1],
                        )
                for t in range(CT):
                    for nb in range(NQ // 512):
                        ps = ps_s.tile([P, 512], f32, tag="s")
                        for i2 in range(2):
                            nc.tensor.matmul(
                                ps,
                                lhsT=w3["wq3"][:, 2 * i2 : 2 * i2 + 2, t * P : (t + 1) * P],
                                rhs=h_all[:, 2 * i2 : 2 * i2 + 2, nb * 512 : (nb + 1) * 512],
                                start=(i2 == 0),
                                stop=(i2 == 1),
                                perf_mode=DR,
                            )
                        nc.scalar.activation(
                            out=q_all[:, t, nb * 512 : (nb + 1) * 512],
                            in_=ps,
                            func=AF.Identity,
                            bias=cols["bqc"][:, t : t + 1],
                        )

                # ---- vT [N, C] ----
                for nb in range(NKB):
                    ps = ps_o.tile([P, C], f32, tag="o")
                    for i2 in range(2):
                        nc.tensor.matmul(
                            ps,
                            lhsT=h_all[:, 2 * i2 : 2 * i2 + 2, nb * P : (nb + 1) * P],
                            rhs=w3["wv3"][:, 2 * i2 : 2 * i2 + 2, :],
                            start=(i2 == 0),
                            stop=(i2 == 1),
                            perf_mode=DR,
                        )
                    nc.vector.tensor_add(out=v_all[:, nb, :], in0=ps, in1=bv_bcast)

                # ---- attention ----
                # Two PE-dense phases per q-chunk with chunk-resident exp
                # tiles: (1) all scores + Exp evictions, (2) all PV + denom
                # matmuls. Chunks are software-pipelined: the next chunk's
                # scores phase is emitted before this chunk's epilogue.
                def scores_phase(qc):
                    qs = qc * 512
                    for j in range(NKB):
                        pss = ps_s.tile([P, 512], f32, tag="s", name="pss")
                        for i2 in range(2):
                            nc.tensor.matmul(
                                pss,
                                lhsT=k_all[:, 2 * i2 : 2 * i2 + 2, j * P : (j + 1) * P],
                                rhs=q_all[:, 2 * i2 : 2 * i2 + 2, qs : qs + 512],
                                start=(i2 == 0),
                                stop=(i2 == 1),
                                perf_mode=DR,
                            )
                        nc.scalar.activation(
                            out=et2[j // 2][:, j % 2, :],
                            in_=pss,
                            func=AF.Exp,
                            scale=SCALE,
                            bias=ebias,
                        )

                def pv_phase():
                    pso = [
                        ps_o.tile([P, 512], f32, tag="o", name="pso")
                        for _ in range(CT)
                    ]
                    psd = ps_d.tile([P, 512], f32, tag="d", name="psd")
                    for jj in range(NKB // 2):
                        for co in range(CT):
                            nc.tensor.matmul(
                                pso[co],
                                lhsT=v_all[:, 2 * jj : 2 * jj + 2, co * P : (co + 1) * P],
                                rhs=et2[jj],
                                start=(jj == 0),
                                stop=(jj == NKB // 2 - 1),
                                perf_mode=DR,
                            )
                        nc.tensor.matmul(
                            psd,
                            lhsT=ones3,
                            rhs=et2[jj],
                            start=(jj == 0),
                            stop=(jj == NKB // 2 - 1),
                            perf_mode=DR,
                        )
                    return pso, psd

                def epilogue(qc, pso, psd):
                    qs = qc * 512
                    for ci in range(CT):
                        nc.vector.tensor_scalar_mul(ao_all[:, ci, :], pso[ci], AO_SCALE)
                    rdb = work.tile([P, 512], f32, tag="rdb", bufs=2)
                    nc.vector.reciprocal(rdb, psd)
                    nc.vector.tensor_scalar_mul(rdb, rdb, 1.0 / AO_SCALE)
                    for co in range(CT):
                        xres = work.tile([P, 512], f32, tag="xres", bufs=4)
                        nc.sync.dma_start(xres, xr_t[co][:, qs : qs + 512])
                        nc.vector.tensor_scalar_add(
                            out=xres, in0=xres, scalar1=cols["boc"][:, co : co + 1]
                        )
                        psp = ps_d.tile([P, 512], f32, tag="d", name="psp")
                        for i2 in range(2):
                            nc.tensor.matmul(
                                psp,
                                lhsT=w3["wo3"][:, 2 * i2 : 2 * i2 + 2, co * P : (co + 1) * P],
                                rhs=ao_all[:, 2 * i2 : 2 * i2 + 2, :],
                                start=(i2 == 0),
                                stop=(i2 == 1),
                                perf_mode=DR,
                            )
                        osb = work.tile([P, 512], f32, tag="osb", bufs=3)
                        nc.vector.tensor_mul(osb, psp, rdb)
                        nc.vector.tensor_add(out=osb, in0=osb, in1=xres)
                        nc.sync.dma_start(out_t[co][:, qs : qs + 512], osb)

                scores_phase(0)
                for qc in range(QCH):
                    pso, psd = pv_phase()
                    if qc + 1 < QCH:
                        scores_phase(qc + 1)
                    epilogue(qc, pso, psd)

    nc.compile()
    return nc


def _get_nc():
    if "nc" not in _cache:
        _cache["nc"] = _build()
    return _cache["nc"]


def _prep_common(inputs):
    fp8 = ml_dtypes.float8_e4m3
    bf16 = ml_dtypes.bfloat16

    def colize(v):
        v = np.asarray(v, np.float32).reshape(CT, P)
        return np.ascontiguousarray(v.T)

    def w3(w):
        # [Cout, Cin] -> [p, cin_tile, cout] fp8 (DoubleRow stationary layout)
        t = np.asarray(w, np.float32).T.reshape(CT, P, C).transpose(1, 0, 2)
        return np.ascontiguousarray(t.astype(fp8))

    gmat = (np.arange(P)[:, None] // 16 == np.arange(8)[None, :]).astype(bf16)

    return {
        "wq3": w3(inputs["wq"]),
        "wk3": w3(inputs["wk"]),
        "wv3": w3(inputs["wv"]),
        "wo3": w3(inputs["wo"]),
        "bqc": colize(inputs["bq"]),
        "bkc": colize(inputs["bk"]),
        "boc": colize(inputs["bo"]),
        "gnwc": colize(inputs["gn_w"]),
        "gnbc": colize(inputs["gn_b"]),
        "bvb": np.ascontiguousarray(
            np.tile(np.asarray(inputs["bv"], np.float32)[None, :], (P, 1))
        ),
        "gmat": gmat,
        "gtmat": np.ascontiguousarray(gmat.T),
    }


def make_in_maps(inputs):
    bf16 = ml_dtypes.bfloat16
    x = np.ascontiguousarray(np.asarray(inputs["hidden_states"], dtype=np.float32))
    B = x.shape[0]
    xs = x.reshape(B, C, N)
    common = _prep_common(inputs)
    in_maps = []
    for core in range(8):
        s, half = core // 2, core % 2
        xc = xs[s] if half == 0 else np.ascontiguousarray(np.roll(xs[s], -NQ, axis=1))
        in_maps.append(
            {
                "xb": np.ascontiguousarray(xc.astype(bf16)),
                "xr": np.ascontiguousarray(xc[:, :NQ]),
                **common,
            }
        )
    return in_maps


def kernel(**inputs):
    from concourse.bass_utils import run_bass_kernel_spmd

    nc = _get_nc()
    in_maps = make_in_maps(inputs)
    res = run_bass_kernel_spmd(nc, in_maps, list(range(8)))

    B = np.asarray(inputs["hidden_states"]).shape[0]
    out = np.empty((B, C, N), np.float32)
    for core in range(8):
        s, half = core // 2, core % 2
        out[s][:, half * NQ : (half + 1) * NQ] = res.results[core]["out"]
    return out.reshape(B, C, 64, 64)
